# revision 1
# baseline (speedup 1.0000x reference)
"""AffNetR TRN2 kernel: out[u,i] = ((max_h cos(Z[h,u,:], X[i,:])) + 1) / 2, ^beta.

Sharding: data-parallel over users (U=8192) across 8 NeuronCores; X replicated.
Each core computes a [1024, 8192] slice of the output.

v2: chunk-streamed prologue. Inputs arrive pre-transposed ([E=128, *]) and are
DMA'd in 512-column chunks (Z first). Per chunk: square on ACT (rounded to
float32r), column-sum-of-squares via an all-ones fp32r matmul into a PSUM row,
then per-round (4 chunks) the [1,2048] row bounces through an HBM scratch to
land partition-major as [4,512]. sqrt(+eps) + reciprocal (DVE) produce the
normalization scales; the X side folds the final /2 affine. One-hot selector
matmuls broadcast each chunk's scale row to 128 partitions, fused into the
operand during PSUM evacuation (DVE tensor_tensor multiply, float32r out).

Main loop per (u-tile 128, i-tile 512): 4 fp32r matmuls (one per head) into 4
PSUM banks; ACT evacuates head 0 adding the +0.5 affine bias; DVE folds heads
1-3 with fused scalar_tensor_tensor ops (out = (psum + 0.5) max chain);
gpsimd-issued DMAs store [128,2048] blocks.

A post-Tile pass splits excess semaphore waits onto inserted NoOps (several
ISA structs only accept one wait slot and walrus rejects more).
"""

import numpy as np

import concourse.bass as bass
import concourse.mybir as mybir
import concourse.tile as tile
from concourse.bass_utils import run_bass_kernel_spmd

F32 = mybir.dt.float32
F32R = mybir.dt.float32r

H = 4
U = 8192
E = 128
I = 8192
NCORES = 8
USH = U // NCORES          # 1024 users per core
UT = USH // 128            # 8 u-tiles
IT = I // 512              # 16 i-tiles
NXC = I // 512             # 16 x chunks of 512
NZC = (H * USH) // 512     # 8 z chunks of 512
EPS = 1e-6

_cache = {}


def _legalize_waits(nc, max_waits=1):
    """Hoist excess sem waits onto same-engine NoOps (1-wait ISA structs)."""
    cnt = 0
    for f in nc.m.functions:
        for blk in f.blocks:
            insts = blk.instructions
            out = []
            changed = False
            for inst in insts:
                si = inst.sync_info
                waits = list(si.on_wait) if si is not None and si.on_wait else []
                if len(waits) > max_waits and inst.engine is not None:
                    keep = waits[-max_waits:]
                    for w in waits[:-max_waits]:
                        nop = mybir.InstNoOp(name=f"wlg-{cnt}", ins=[], outs=[])
                        cnt += 1
                        nop.engine = inst.engine
                        nop.sync_info = mybir.SyncInfo(on_wait=[w], on_update=[])
                        out.append(nop)
                    upd = list(si.on_update) if si.on_update else []
                    inst.sync_info = mybir.SyncInfo(on_wait=keep, on_update=upd)
                    changed = True
                out.append(inst)
            if changed:
                blk.instructions = out
    return cnt


def _build():
    nc = bass.Bass()
    xt_d = nc.dram_tensor("xt", [E, I], F32, kind="ExternalInput")
    zt_d = nc.dram_tensor("zt", [E, H * USH], F32, kind="ExternalInput")
    sel_d = nc.dram_tensor("sel", [16, 16 * 128], F32R, kind="ExternalInput")
    out_d = nc.dram_tensor("out", [USH, I], F32, kind="ExternalOutput")
    scr_d = nc.dram_tensor("scr", [12, 1024], F32)
    out_v = out_d[:].rearrange("(uo p) i -> p uo i", p=128)

    S = mybir.ActivationFunctionType

    with tile.TileContext(nc) as tc:
        with tc.tile_pool(name="big", bufs=1) as big:
            pre_ctx = tc.tile_pool(name="pre", bufs=1)
            pre = pre_ctx.__enter__()
            xt_sb = pre.tile([E, I], F32, tag="xt_sb")
            zt_sb = pre.tile([E, H * USH], F32, tag="zt_sb")
            sqx = pre.tile([E, I], F32R, tag="sqx")
            sqz = pre.tile([E, H * USH], F32R, tag="sqz")
            rxg = pre.tile([16, 512], F32, tag="rxg")
            rzg = pre.tile([8, 512], F32, tag="rzg")
            sel_r = big.tile([16, 16 * 128], F32R, tag="sel_r")
            xtn = big.tile([E, I], F32R, tag="xtn")
            ztn = big.tile([E, H * USH], F32R, tag="ztn")
            rx05 = big.tile([16, 512], F32R, tag="rx05")
            rz1 = big.tile([8, 512], F32R, tag="rz1")

            # input DMAs, Z chunks first (Z gates the main loop's lhsT)
            for c in range(NZC):
                s = slice(c * 512, (c + 1) * 512)
                nc.sync.dma_start(zt_sb[:, s], zt_d[:, s])
            for c in range(NXC):
                s = slice(c * 512, (c + 1) * 512)
                nc.sync.dma_start(xt_sb[:, s], xt_d[:, s])
            nc.sync.dma_start(sel_r, sel_d[:])

            onesf = big.tile([128, 1], F32, tag="onesf")
            nc.vector.memset(onesf, 1.0)
            ones_r = big.tile([128, 1], F32R, tag="ones_r")
            nc.scalar.copy(ones_r, onesf)
            half1 = big.tile([128, 1], F32, tag="half1")
            nc.vector.memset(half1, 0.5)

            rows_ctx = tc.tile_pool(name="rows", bufs=4)
            rows_pool = rows_ctx.__enter__()
            pcols_ctx = tc.tile_pool(name="pcols", bufs=2, space="PSUM")
            pcols = pcols_ctx.__enter__()
            prep_ctx = tc.tile_pool(name="prep", bufs=4, space="PSUM")
            prep = prep_ctx.__enter__()

            def colsum_rounds(src, sq, nchunks, scr0, scat):
                """Square chunks, column-sum via all-ones matmul, bounce each
                [1,1024] round through HBM to land as [2,512] partition rows."""
                for rnd in range(nchunks // 2):
                    ss = pcols.tile([1, 1024], F32, tag="ss")
                    for j in range(2):
                        c = rnd * 2 + j
                        s = slice(c * 512, (c + 1) * 512)
                        nc.scalar.activation(sq[:, s], src[:, s], S.Square)
                        nc.tensor.matmul(
                            ss[:, j * 512 : (j + 1) * 512],
                            ones_r,
                            sq[:, s],
                            start=True,
                            stop=True,
                        )
                    row = rows_pool.tile([1, 1024], F32, tag="row")
                    nc.vector.tensor_copy(row, ss)
                    k = scr0 + rnd
                    nc.gpsimd.dma_start(scr_d[k : k + 1, :], row)
                    nc.sync.dma_start(
                        scat[rnd * 2 : (rnd + 1) * 2, :],
                        scr_d[k, :].rearrange("(c n) -> c n", c=2),
                    )

            def norm_chain(g, n_par, scale, out_r):
                nrm = pre.tile([n_par, 512], F32, tag=f"nrm{n_par}")
                nc.scalar.activation(nrm, g, S.Sqrt)
                ne = pre.tile([n_par, 512], F32, tag=f"ne{n_par}")
                nc.vector.tensor_scalar_add(ne, nrm, EPS)
                rr = pre.tile([n_par, 512], F32, tag=f"rr{n_par}")
                nc.vector.reciprocal(rr, ne)
                nc.vector.tensor_scalar_mul(out_r, rr, scale)

            def replicate_evac(cs, kk, r_in, src, dst):
                for c in cs:
                    s = slice(c * 512, (c + 1) * 512)
                    rep = prep.tile([128, 512], F32, tag="rep")
                    nc.tensor.matmul(
                        rep,
                        sel_r[0:kk, c * 128 : (c + 1) * 128],
                        r_in,
                        start=True,
                        stop=True,
                    )
                    nc.vector.tensor_tensor(
                        dst[:, s], src[:, s], rep, mybir.AluOpType.mult
                    )

            # colsums first, then chains, then replicates (v5 order)
            colsum_rounds(zt_sb, sqz, NZC, 0, rzg)
            colsum_rounds(xt_sb, sqx, NXC, 4, rxg)
            norm_chain(rzg, 8, 1.0, rz1)
            norm_chain(rxg, 16, 0.5, rx05)
            replicate_evac(range(NZC), 8, rz1, zt_sb, ztn)
            replicate_evac(range(NXC), 16, rx05, xt_sb, xtn)

            prep_ctx.__exit__(None, None, None)
            pcols_ctx.__exit__(None, None, None)
            rows_ctx.__exit__(None, None, None)
            pre_ctx.__exit__(None, None, None)

            # ---------- main loop ----------
            with (
                tc.tile_pool(name="work", bufs=3) as work,
                tc.tile_pool(name="ost", bufs=2) as ost,
                tc.tile_pool(name="pmm", bufs=2, space="PSUM") as pmm,
            ):
                for ut in range(UT):
                    lhs = [
                        ztn[:, h * USH + ut * 128 : h * USH + (ut + 1) * 128]
                        for h in range(H)
                    ]
                    for it in range(IT):
                        rhs = xtn[:, it * 512 : (it + 1) * 512]
                        ps = []
                        for h in range(H):
                            p = pmm.tile([128, 512], F32, tag=f"p{h}")
                            nc.tensor.matmul(p, lhs[h], rhs, start=True, stop=True)
                            ps.append(p)
                        c0 = work.tile([128, 512], F32, tag="c0")
                        nc.scalar.activation(
                            c0, ps[0], S.Identity, bias=half1, scale=1.0
                        )
                        m1 = work.tile([128, 512], F32, tag="m1")
                        nc.vector.scalar_tensor_tensor(
                            m1, ps[1], 0.5, c0,
                            op0=mybir.AluOpType.add, op1=mybir.AluOpType.max,
                        )
                        m2 = work.tile([128, 512], F32, tag="m2")
                        nc.vector.scalar_tensor_tensor(
                            m2, ps[2], 0.5, m1,
                            op0=mybir.AluOpType.add, op1=mybir.AluOpType.max,
                        )
                        if it % 4 == 0:
                            ostage = ost.tile([128, 2048], F32, tag="ostage")
                        nc.vector.scalar_tensor_tensor(
                            ostage[:, (it % 4) * 512 : (it % 4 + 1) * 512],
                            ps[3], 0.5, m2,
                            op0=mybir.AluOpType.add, op1=mybir.AluOpType.max,
                        )
                        if ut == UT - 1 and it >= 12:
                            j = it % 4
                            nc.gpsimd.dma_start(
                                out_v[:, ut, (12 + j) * 512 : (13 + j) * 512],
                                ostage[:, j * 512 : (j + 1) * 512],
                            )
                        elif it % 4 == 3:
                            ig = it // 4
                            nc.gpsimd.dma_start(
                                out_v[:, ut, ig * 2048 : (ig + 1) * 2048],
                                ostage,
                            )

    _legalize_waits(nc)
    return nc


def _sel_host():
    sel = np.zeros((16, 16 * 128), dtype=np.float32)
    for c in range(16):
        sel[c, c * 128 : (c + 1) * 128] = 1.0
    return sel


def kernel(X, Z, beta):
    X = np.asarray(X, dtype=np.float32)
    Z = np.asarray(Z, dtype=np.float32)
    xt = np.ascontiguousarray(X.T)                      # [128, 8192]
    sel = _sel_host()
    in_maps = []
    for c in range(NCORES):
        zs = Z[:, c * USH : (c + 1) * USH, :]           # [4, 1024, 128]
        zt = np.ascontiguousarray(
            zs.transpose(2, 0, 1).reshape(E, H * USH)
        )                                               # [128, 4096]
        in_maps.append({"xt": xt, "zt": zt, "sel": sel})

    if "nc" not in _cache:
        _cache["nc"] = _build()
    res = run_bass_kernel_spmd(_cache["nc"], in_maps, list(range(NCORES))).results
    out = np.concatenate([r["out"] for r in res], axis=0)

    b = float(np.asarray(beta))
    if b != 1.0:
        out = np.power(out, b).astype(np.float32)
    return out

